# revision 26
# baseline (speedup 1.0000x reference)
"""DeltaTokenShift Trainium2 kernel (Bass/Tile, 8 NeuronCores via axon).

Computation (per batch b):
    erase = sigmoid(x @ We + be) ; write = sigmoid(x @ Ww + bw)
    s_t = s_{t-1} * (1 - erase_t) + write_t * x_t   (scan over L, per channel)
    out[:, t, :] = s_t

Sharding: 8 cores = 4 batches x 2 halves of the 1024-channel dim. Each core
gets the full x[b] (the gate matmul contracts over all 1024 input channels),
its 512-column weight slices, bias/state slices, and computes
out[b][:, half] = [4096, 512]. For upper-half cores, x columns and weight
rows are rotated by 512 on the host so the core's own gate channels always
occupy xT k-tiles 0..3 (a consistent permutation of the contraction dim
leaves the matmul result unchanged).

All layout work is host-side. x ships transposed, bf16, and packed in
k-planes: xT[p, k, l] = x[l, 128k+p] (so the contraction dim sits on
partitions with no PE transposes, and one DMA issue can move any k-range).
Weights likewise pack to [128, k, e] bf16; the kernel writes
outT[p, m, l] = out[l, 128m+p], unpacked on the host. bf16 gate inputs
keep the PE at 1 col/cycle (same as f32r) while halving HBM traffic;
fp8 was measured out of tolerance (2.6e-2) and bf16 in (3.3e-3).

The PE then runs ONLY the 512 gate matmuls [128e, 512l] = 8-step bf16
accumulations in PSUM, which is the compute floor for this op. Per
512-token chunk, per m-group: ACT sigmoid straight from PSUM (erase uses
scale=-1, bias=-be => a = 1-sigmoid), GpSimd b = write * x_f32 (x cast
hoisted to block top on DVE), DVE tensor_tensor_scan(a, b) chained via
initial=prev[:, -1:], per-m DMA of outT block slabs.

Ramp/tail details: 13 warmup matmuls on an iota-filled tile bridge the
DMA preload so the PE DVFS ladder (~0.8 -> ~1.35 -> 2.4GHz, ~3.4us of
continuous busy per step; zeros do NOT ramp it - no switching activity)
is climbed before real work; the preload is 2 k-group DMA issues per
tensor (per-issue cost ~650ns makes fine-grained preloads issue-bound);
the first chunk is erase-gate-major (write weights still streaming); the
last chunk is write-gate-major with b=w*x precomputed, so the post-last-
matmul chain is only sigmoid -> scan -> DMA.

Measured: ~133-136us HW exec (machine has a fast and a ~1.2x slower
whole-chip clock state), rel err 3.3e-3 vs fp32 CPU reference.
"""

import sys

sys.path.insert(0, "/opt/trn_rl_repo")

import numpy as np
import concourse.bacc as bacc
import concourse.mybir as mybir
from concourse.tile import TileContext
from concourse.bass_utils import run_bass_kernel_spmd

B, L = 4, 4096

F32 = mybir.dt.float32
F32R = mybir.dt.float32r
BF16 = mybir.dt.bfloat16

P = 128
DIN = 1024
ESH = 512
KT = DIN // P  # 8 contraction k-tiles
MT = ESH // P  # 4 output-channel groups per core


def _build_kernel_impl(L=4096, blocks=None, warmup=0, psum_bufs=4,
                       out_bf16=False,
                       preload_groups=((0,), (1,), (2, 3), (4, 5, 6, 7)),
                       w_queue="sync", x_queue="sync", o_queue="sync"):
    """blocks: list of lists of chunk widths; each inner list is one DMA
    block (xT slab in, outT slab out). preload_groups: k-ranges, one DMA
    issue each, for the weight + block-0 x preload."""
    if blocks is None:
        blocks = [[512], [512, 512], [512, 512], [512, 512], [512]]
    assert sum(sum(b) for b in blocks) == L
    lbmax = max(sum(b) for b in blocks)
    assert sorted(k for g in preload_groups for k in g) == list(range(KT))
    for g in preload_groups:
        assert list(g) == list(range(g[0], g[0] + len(g)))

    nc = bacc.Bacc("TRN2", target_bir_lowering=False)
    queues = {"sync": nc.sync, "scalar": nc.scalar, "vector": nc.vector,
              "gpsimd": nc.gpsimd}
    wq, xq, oq = queues[w_queue], queues[x_queue], queues[o_queue]

    xT = nc.dram_tensor("xT", [P, KT, L], BF16, kind="ExternalInput")
    we = nc.dram_tensor("we", [P, KT, ESH], BF16, kind="ExternalInput")
    ww = nc.dram_tensor("ww", [P, KT, ESH], BF16, kind="ExternalInput")
    # biases[:, m] = -erase_bias group m ; biases[:, MT+m] = +write_bias group m
    biases = nc.dram_tensor("biases", [P, 2 * MT], F32, kind="ExternalInput")
    state0 = nc.dram_tensor("state0", [P, MT], F32, kind="ExternalInput")
    odt = BF16 if out_bf16 else F32
    outT = nc.dram_tensor("outT", [P, MT, L], odt, kind="ExternalOutput")

    with TileContext(nc) as tc:
        with (
            tc.tile_pool(name="const", bufs=1) as constp,
            tc.tile_pool(name="wsb", bufs=1) as wsb,
            tc.tile_pool(name="xt", bufs=2) as xtp,
            tc.tile_pool(name="gate", bufs=4) as gatep,
            tc.tile_pool(name="bmul", bufs=3) as bmulp,
            tc.tile_pool(name="scan", bufs=2) as scanp,
            tc.tile_pool(name="ps_mm", bufs=psum_bufs, space="PSUM") as ps_mm,
        ):
            if warmup:
                # Dummy matmuls keep the PE busy while the first DMAs
                # land. Random operand data (not zeros): the DVFS governor
                # responds to switching activity, so zero-valued warmups
                # leave the clock low.
                wu_sb = constp.tile([P, ESH], mybir.dt.int16, tag="wu")
                nc.gpsimd.iota(wu_sb[:], [[1, ESH]], channel_multiplier=997,
                               allow_small_or_imprecise_dtypes=True)
                wu_ps = [ps_mm.tile([P, ESH], F32, tag=f"wups{j}",
                                    name=f"wups{j}", bufs=1)
                         for j in range(2)]
                wu_b = wu_sb[:].bitcast(BF16)
                for j in range(warmup):
                    nc.tensor.matmul(
                        wu_ps[j % 2][:], wu_b[:, :P], wu_b,
                        start=True, stop=True, skip_group_check=True)

            # Preload: erase weights + block-0 x first (the first chunk is
            # emitted gate-major, so the PE only needs we+x to start), then
            # the write weights, which land during the erase sweep.
            lb0 = sum(blocks[0])
            w_sb = [wsb.tile([P, KT, ESH], BF16, tag=f"w{gi}", name=f"w{gi}")
                    for gi in range(2)]
            xt0 = xtp.tile([P, KT, lbmax], BF16, tag="xt", name="xt")
            for g in preload_groups:
                ks = slice(g[0], g[-1] + 1)
                wq.dma_start(w_sb[0][:, ks, :], we[:, ks, :])
                xq.dma_start(xt0[:, ks, :lb0], xT[:, ks, :lb0])
            for g in preload_groups:
                ks = slice(g[0], g[-1] + 1)
                wq.dma_start(w_sb[1][:, ks, :], ww[:, ks, :])

            bias_sb = constp.tile([P, 2 * MT], F32, tag="bias")
            nc.sync.dma_start(bias_sb[:], biases[:])
            st_sb = constp.tile([P, MT], F32, tag="st")
            nc.sync.dma_start(st_sb[:], state0[:])

            prev_s = [None] * MT
            b0 = 0

            for blki, chunks in enumerate(blocks):
                lb = sum(chunks)
                if blki == 0:
                    xt = xt0
                else:
                    xt = xtp.tile([P, KT, lbmax], BF16, tag="xt", name="xt")
                    xq.dma_start(xt[:, :, :lb], xT[:, :, b0:b0 + lb])

                # Hoist the f32 casts of the b-term x slabs out of the
                # per-m critical chain: they only depend on the xT DMA.
                xf = [None] * MT
                for m in range(MT):
                    xf[m] = bmulp.tile([P, lbmax], F32, tag=f"xf{m}",
                                       name=f"xf{m}", bufs=2)
                    nc.vector.tensor_copy(xf[m][:, :lb], xt[:, m, :lb])

                def mm_group(gi, m, lo, lc):
                    ps = ps_mm.tile([P, ESH], F32, tag="psmm", name="psmm")
                    for k in range(KT):
                        nc.tensor.matmul(
                            ps[:, :lc],
                            w_sb[gi][:, k, m * P:(m + 1) * P],
                            xt[:, k, lo:lo + lc],
                            start=(k == 0), stop=(k == KT - 1),
                        )
                    g_t = gatep.tile([P, ESH], F32, tag="aw"[gi],
                                     name="aw"[gi])
                    nc.scalar.activation(
                        g_t[:, :lc], ps[:, :lc],
                        mybir.ActivationFunctionType.Sigmoid,
                        bias=bias_sb[:, gi * MT + m:gi * MT + m + 1],
                        scale=-1.0 if gi == 0 else 1.0,
                    )
                    return g_t

                s_tiles = [None] * MT
                lo = 0
                for ci, lc in enumerate(chunks):
                    last_chunk = (blki == len(blocks) - 1
                                  and ci == len(chunks) - 1)
                    gates = [[None] * MT, [None] * MT]
                    if blki == 0 and ci == 0:
                        # Gate-major: the erase groups only need the we
                        # tiles, which land first during the preload.
                        for gi in range(2):
                            for m in range(MT):
                                gates[gi][m] = mm_group(gi, m, lo, lc)
                    elif last_chunk:
                        # Write-gate-major: the b = w*x mults complete
                        # while the erase groups still run, so the tail
                        # chain after the final matmul is only
                        # sigmoid -> scan -> DMA.
                        for gi in (1, 0):
                            for m in range(MT):
                                gates[gi][m] = mm_group(gi, m, lo, lc)
                        for m in range(MT):
                            b_t = bmulp.tile([P, ESH], F32, tag="b")
                            nc.gpsimd.tensor_tensor(
                                b_t[:, :lc], gates[1][m][:, :lc],
                                xf[m][:, lo:lo + lc],
                                op=mybir.AluOpType.mult)
                            gates[1][m] = b_t
                    else:
                        for m in range(MT):
                            gates[0][m] = mm_group(0, m, lo, lc)
                            gates[1][m] = mm_group(1, m, lo, lc)

                    for m in range(MT):
                        a_t = gates[0][m]
                        if last_chunk:
                            b_t = gates[1][m]  # already w*x
                        else:
                            b_t = bmulp.tile([P, ESH], F32, tag="b")
                            # GpSimd is otherwise idle; fully parallel with
                            # DVE, and operands + out are SBUF (P2-safe).
                            nc.gpsimd.tensor_tensor(
                                b_t[:, :lc], gates[1][m][:, :lc],
                                xf[m][:, lo:lo + lc],
                                op=mybir.AluOpType.mult)

                        if ci == 0:
                            s_tiles[m] = scanp.tile(
                                [P, lbmax], odt, tag=f"s{m}", name=f"s{m}")
                            init = st_sb[:, m:m + 1] if blki == 0 else \
                                prev_s[m][:, prev_lb - 1:prev_lb]
                        else:
                            init = s_tiles[m][:, lo - 1:lo]
                        nc.vector.tensor_tensor_scan(
                            s_tiles[m][:, lo:lo + lc], a_t[:, :lc],
                            b_t[:, :lc], init,
                            op0=mybir.AluOpType.mult, op1=mybir.AluOpType.add,
                        )
                        if ci == len(chunks) - 1:
                            oq.dma_start(
                                outT[:, m, b0:b0 + lb], s_tiles[m][:, :lb])
                            prev_s[m] = s_tiles[m]
                    lo += lc
                prev_lb = lb
                b0 += lb

    nc.finalize()
    return nc


_cached_nc = None


def _build_kernel():
    return _build_kernel_impl(
        L=L, warmup=13, preload_groups=((0, 1, 2, 3), (4, 5, 6, 7)))


def _shard_inputs(x, state, erase_kernel, erase_bias, write_kernel, write_bias):
    import ml_dtypes
    bf16 = ml_dtypes.bfloat16

    def pack_k(a2d):  # [DIN, C] -> [P, KT, C]
        return np.ascontiguousarray(
            a2d.reshape(KT, P, a2d.shape[1]).transpose(1, 0, 2))

    maps = []
    for core in range(8):
        b, h = divmod(core, 2)
        e0 = h * ESH
        xb = x[b]
        web = erase_kernel[:, e0:e0 + ESH]
        wwb = write_kernel[:, e0:e0 + ESH]
        if h == 1:
            xb = np.concatenate([xb[:, ESH:], xb[:, :ESH]], axis=1)
            web = np.concatenate([web[ESH:, :], web[:ESH, :]], axis=0)
            wwb = np.concatenate([wwb[ESH:, :], wwb[:ESH, :]], axis=0)
        ben = (-erase_bias[e0:e0 + ESH]).reshape(MT, P).T
        bwp = write_bias[e0:e0 + ESH].reshape(MT, P).T
        stp = state[b, e0:e0 + ESH].reshape(MT, P).T
        maps.append({
            "xT": pack_k(np.ascontiguousarray(xb.T).astype(bf16)),
            "we": pack_k(web.astype(bf16)),
            "ww": pack_k(wwb.astype(bf16)),
            "biases": np.ascontiguousarray(
                np.concatenate([ben, bwp], axis=1), dtype=np.float32),
            "state0": np.ascontiguousarray(stp, dtype=np.float32),
        })
    return maps


def kernel(x, state, erase_kernel, erase_bias, write_kernel, write_bias):
    global _cached_nc
    x = np.asarray(x, np.float32)
    state = np.asarray(state, np.float32)
    erase_kernel = np.asarray(erase_kernel, np.float32)
    erase_bias = np.asarray(erase_bias, np.float32)
    write_kernel = np.asarray(write_kernel, np.float32)
    write_bias = np.asarray(write_bias, np.float32)

    if _cached_nc is None:
        _cached_nc = _build_kernel()
    maps = _shard_inputs(x, state, erase_kernel, erase_bias,
                         write_kernel, write_bias)
    res = run_bass_kernel_spmd(_cached_nc, maps, core_ids=list(range(8)))
    full = np.empty((B, L, DIN), np.float32)
    for core in range(8):
        b, h = divmod(core, 2)
        o = res.results[core]["outT"]  # [P, MT, L]
        full[b, :, h * ESH:(h + 1) * ESH] = \
            o.transpose(2, 1, 0).reshape(L, ESH)
    return full


# revision 29
# speedup vs baseline: 1.0362x; 1.0362x over previous
"""DeltaTokenShift Trainium2 kernel (Bass/Tile, 8 NeuronCores via axon).

Computation (per batch b):
    erase = sigmoid(x @ We + be) ; write = sigmoid(x @ Ww + bw)
    s_t = s_{t-1} * (1 - erase_t) + write_t * x_t   (scan over L, per channel)
    out[:, t, :] = s_t

Sharding: 8 cores = 4 batches x 2 halves of the 1024-channel dim. Each core
gets the full x[b] (the gate matmul contracts over all 1024 input channels),
its 512-column weight slices, bias/state slices, and computes
out[b][:, half] = [4096, 512]. For upper-half cores, x columns and weight
rows are rotated by 512 on the host so the core's own gate channels always
occupy xT k-tiles 0..3 (a consistent permutation of the contraction dim
leaves the matmul result unchanged).

All layout work is host-side. x ships transposed, bf16, and packed in
k-planes: xT[p, k, l] = x[l, 128k+p] (so the contraction dim sits on
partitions with no PE transposes, and one DMA issue can move any k-range).
Weights likewise pack to [128, k, e] bf16; the kernel writes
outT[p, m, l] = out[l, 128m+p], unpacked on the host. bf16 gate inputs
keep the PE at 1 col/cycle (same as f32r) while halving HBM traffic;
fp8 was measured out of tolerance (2.6e-2) and bf16 in (3.3e-3).

The PE then runs ONLY the 512 gate matmuls [128e, 512l] = 8-step bf16
accumulations in PSUM, which is the compute floor for this op. Per
512-token chunk, per m-group: ACT sigmoid straight from PSUM (erase uses
scale=-1, bias=-be => a = 1-sigmoid), GpSimd b = write * x_f32 (x cast
hoisted to block top on DVE), DVE tensor_tensor_scan(a, b) chained via
initial=prev[:, -1:], per-m DMA of outT block slabs.

Ramp/tail details: 13 warmup matmuls on an iota-filled tile bridge the
DMA preload so the PE DVFS ladder (~0.8 -> ~1.35 -> 2.4GHz, ~3.4us of
continuous busy per step; zeros do NOT ramp it - no switching activity)
is climbed before real work; the preload is 2 k-group DMA issues per
tensor (per-issue cost ~650ns makes fine-grained preloads issue-bound);
the first chunk is erase-gate-major (write weights still streaming); the
last chunk is write-gate-major with b=w*x precomputed, so the post-last-
matmul chain is only sigmoid -> scan -> DMA.

Measured: ~133-136us HW exec (machine has a fast and a ~1.2x slower
whole-chip clock state), rel err 3.3e-3 vs fp32 CPU reference.
"""

import sys

sys.path.insert(0, "/opt/trn_rl_repo")

import numpy as np
import concourse.bacc as bacc
import concourse.mybir as mybir
from concourse.tile import TileContext
from concourse.bass_utils import run_bass_kernel_spmd

B, L = 4, 4096

F32 = mybir.dt.float32
F32R = mybir.dt.float32r
BF16 = mybir.dt.bfloat16

P = 128
DIN = 1024
ESH = 512
KT = DIN // P  # 8 contraction k-tiles
MT = ESH // P  # 4 output-channel groups per core


def _build_kernel_impl(L=4096, blocks=None, warmup=0, psum_bufs=4,
                       out_bf16=False, kmajor_first=False,
                       preload_groups=((0,), (1,), (2, 3), (4, 5, 6, 7)),
                       w_queue="sync", x_queue="sync", o_queue="sync"):
    """blocks: list of lists of chunk widths; each inner list is one DMA
    block (xT slab in, outT slab out). preload_groups: k-ranges, one DMA
    issue each, for the weight + block-0 x preload."""
    if blocks is None:
        blocks = [[512], [512, 512], [512, 512], [512, 512], [512]]
    assert sum(sum(b) for b in blocks) == L
    lbmax = max(sum(b) for b in blocks)
    assert sorted(k for g in preload_groups for k in g) == list(range(KT))
    for g in preload_groups:
        assert list(g) == list(range(g[0], g[0] + len(g)))

    nc = bacc.Bacc("TRN2", target_bir_lowering=False)
    queues = {"sync": nc.sync, "scalar": nc.scalar, "vector": nc.vector,
              "gpsimd": nc.gpsimd}
    wq, xq, oq = queues[w_queue], queues[x_queue], queues[o_queue]

    xT = nc.dram_tensor("xT", [P, KT, L], BF16, kind="ExternalInput")
    we = nc.dram_tensor("we", [P, KT, ESH], BF16, kind="ExternalInput")
    ww = nc.dram_tensor("ww", [P, KT, ESH], BF16, kind="ExternalInput")
    # biases[:, m] = -erase_bias group m ; biases[:, MT+m] = +write_bias group m
    biases = nc.dram_tensor("biases", [P, 2 * MT], F32, kind="ExternalInput")
    state0 = nc.dram_tensor("state0", [P, MT], F32, kind="ExternalInput")
    odt = BF16 if out_bf16 else F32
    outT = nc.dram_tensor("outT", [P, MT, L], odt, kind="ExternalOutput")

    with TileContext(nc) as tc:
        with (
            tc.tile_pool(name="const", bufs=1) as constp,
            tc.tile_pool(name="wsb", bufs=1) as wsb,
            tc.tile_pool(name="xt", bufs=2) as xtp,
            tc.tile_pool(name="gate", bufs=4) as gatep,
            tc.tile_pool(name="bmul", bufs=3) as bmulp,
            tc.tile_pool(name="scan", bufs=2) as scanp,
            tc.tile_pool(name="ps_mm", bufs=psum_bufs, space="PSUM") as ps_mm,
        ):
            if warmup:
                # Dummy matmuls keep the PE busy while the first DMAs
                # land. Random operand data (not zeros): the DVFS governor
                # responds to switching activity, so zero-valued warmups
                # leave the clock low.
                wu_sb = constp.tile([P, ESH], mybir.dt.int16, tag="wu")
                nc.gpsimd.iota(wu_sb[:], [[1, ESH]], channel_multiplier=997,
                               allow_small_or_imprecise_dtypes=True)
                wu_ps = [ps_mm.tile([P, ESH], F32, tag=f"wups{j}",
                                    name=f"wups{j}", bufs=1)
                         for j in range(2)]
                wu_b = wu_sb[:].bitcast(BF16)
                for j in range(warmup):
                    nc.tensor.matmul(
                        wu_ps[j % 2][:], wu_b[:, :P], wu_b,
                        start=True, stop=True, skip_group_check=True)

            # Preload: erase weights + block-0 x first (the first chunk is
            # emitted gate-major, so the PE only needs we+x to start), then
            # the write weights, which land during the erase sweep.
            lb0 = sum(blocks[0])
            w_sb = [wsb.tile([P, KT, ESH], BF16, tag=f"w{gi}", name=f"w{gi}")
                    for gi in range(2)]
            xt0 = xtp.tile([P, KT, lbmax], BF16, tag="xt", name="xt")
            if kmajor_first:
                # k-major first chunk consumes both gates' k-slabs in
                # stream order, so interleave ww per group too.
                for g in preload_groups:
                    ks = slice(g[0], g[-1] + 1)
                    wq.dma_start(w_sb[0][:, ks, :], we[:, ks, :])
                    wq.dma_start(w_sb[1][:, ks, :], ww[:, ks, :])
                    xq.dma_start(xt0[:, ks, :lb0], xT[:, ks, :lb0])
            else:
                for g in preload_groups:
                    ks = slice(g[0], g[-1] + 1)
                    wq.dma_start(w_sb[0][:, ks, :], we[:, ks, :])
                    xq.dma_start(xt0[:, ks, :lb0], xT[:, ks, :lb0])
                for g in preload_groups:
                    ks = slice(g[0], g[-1] + 1)
                    wq.dma_start(w_sb[1][:, ks, :], ww[:, ks, :])

            bias_sb = constp.tile([P, 2 * MT], F32, tag="bias")
            nc.sync.dma_start(bias_sb[:], biases[:])
            st_sb = constp.tile([P, MT], F32, tag="st")
            nc.sync.dma_start(st_sb[:], state0[:])

            prev_s = [None] * MT
            b0 = 0

            for blki, chunks in enumerate(blocks):
                lb = sum(chunks)
                if blki == 0:
                    xt = xt0
                else:
                    xt = xtp.tile([P, KT, lbmax], BF16, tag="xt", name="xt")
                    xq.dma_start(xt[:, :, :lb], xT[:, :, b0:b0 + lb])

                # Hoist the f32 casts of the b-term x slabs out of the
                # per-m critical chain: they only depend on the xT DMA.
                xf = [None] * MT
                for m in range(MT):
                    xf[m] = bmulp.tile([P, lbmax], F32, tag=f"xf{m}",
                                       name=f"xf{m}", bufs=2)
                    nc.vector.tensor_copy(xf[m][:, :lb], xt[:, m, :lb])

                def mm_group(gi, m, lo, lc):
                    ps = ps_mm.tile([P, ESH], F32, tag="psmm", name="psmm")
                    for k in range(KT):
                        nc.tensor.matmul(
                            ps[:, :lc],
                            w_sb[gi][:, k, m * P:(m + 1) * P],
                            xt[:, k, lo:lo + lc],
                            start=(k == 0), stop=(k == KT - 1),
                        )
                    g_t = gatep.tile([P, ESH], F32, tag="aw"[gi],
                                     name="aw"[gi])
                    nc.scalar.activation(
                        g_t[:, :lc], ps[:, :lc],
                        mybir.ActivationFunctionType.Sigmoid,
                        bias=bias_sb[:, gi * MT + m:gi * MT + m + 1],
                        scale=-1.0 if gi == 0 else 1.0,
                    )
                    return g_t

                s_tiles = [None] * MT
                lo = 0
                for ci, lc in enumerate(chunks):
                    last_chunk = (blki == len(blocks) - 1
                                  and ci == len(chunks) - 1)
                    gates = [[None] * MT, [None] * MT]
                    if blki == 0 and ci == 0 and kmajor_first:
                        # k-major in two halves of 4 PSUM groups each: the
                        # PE consumes every k-slab as it streams in instead
                        # of head-of-line blocking on a later k while an
                        # earlier one still has runnable matmuls.
                        for ms in ((0, 1), (2, 3)):
                            pss = {}
                            for m in ms:
                                for gi in range(2):
                                    pss[(gi, m)] = ps_mm.tile(
                                        [P, ESH], F32, tag="psmm",
                                        name="psmm")
                            for k in range(KT):
                                for m in ms:
                                    for gi in range(2):
                                        nc.tensor.matmul(
                                            pss[(gi, m)][:, :lc],
                                            w_sb[gi][:, k,
                                                     m * P:(m + 1) * P],
                                            xt[:, k, lo:lo + lc],
                                            start=(k == 0),
                                            stop=(k == KT - 1),
                                        )
                            for m in ms:
                                for gi in range(2):
                                    g_t = gatep.tile(
                                        [P, ESH], F32, tag="aw"[gi],
                                        name="aw"[gi])
                                    nc.scalar.activation(
                                        g_t[:, :lc], pss[(gi, m)][:, :lc],
                                        mybir.ActivationFunctionType.Sigmoid,
                                        bias=bias_sb[
                                            :, gi * MT + m:gi * MT + m + 1],
                                        scale=-1.0 if gi == 0 else 1.0,
                                    )
                                    gates[gi][m] = g_t
                    elif blki == 0 and ci == 0:
                        # Gate-major: the erase groups only need the we
                        # tiles, which land first during the preload.
                        for gi in range(2):
                            for m in range(MT):
                                gates[gi][m] = mm_group(gi, m, lo, lc)
                    elif last_chunk:
                        # Write-gate-major: the b = w*x mults complete
                        # while the erase groups still run, so the tail
                        # chain after the final matmul is only
                        # sigmoid -> scan -> DMA.
                        for gi in (1, 0):
                            for m in range(MT):
                                gates[gi][m] = mm_group(gi, m, lo, lc)
                        for m in range(MT):
                            b_t = bmulp.tile([P, ESH], F32, tag="b")
                            nc.gpsimd.tensor_tensor(
                                b_t[:, :lc], gates[1][m][:, :lc],
                                xf[m][:, lo:lo + lc],
                                op=mybir.AluOpType.mult)
                            gates[1][m] = b_t
                    else:
                        for m in range(MT):
                            gates[0][m] = mm_group(0, m, lo, lc)
                            gates[1][m] = mm_group(1, m, lo, lc)

                    for m in range(MT):
                        a_t = gates[0][m]
                        if last_chunk:
                            b_t = gates[1][m]  # already w*x
                        else:
                            b_t = bmulp.tile([P, ESH], F32, tag="b")
                            # GpSimd is otherwise idle; fully parallel with
                            # DVE, and operands + out are SBUF (P2-safe).
                            nc.gpsimd.tensor_tensor(
                                b_t[:, :lc], gates[1][m][:, :lc],
                                xf[m][:, lo:lo + lc],
                                op=mybir.AluOpType.mult)

                        if ci == 0:
                            s_tiles[m] = scanp.tile(
                                [P, lbmax], odt, tag=f"s{m}", name=f"s{m}")
                            init = st_sb[:, m:m + 1] if blki == 0 else \
                                prev_s[m][:, prev_lb - 1:prev_lb]
                        else:
                            init = s_tiles[m][:, lo - 1:lo]
                        nc.vector.tensor_tensor_scan(
                            s_tiles[m][:, lo:lo + lc], a_t[:, :lc],
                            b_t[:, :lc], init,
                            op0=mybir.AluOpType.mult, op1=mybir.AluOpType.add,
                        )
                        if ci == len(chunks) - 1:
                            oq.dma_start(
                                outT[:, m, b0:b0 + lb], s_tiles[m][:, :lb])
                            prev_s[m] = s_tiles[m]
                    lo += lc
                prev_lb = lb
                b0 += lb

    nc.finalize()
    return nc


_cached_nc = None


def _build_kernel():
    return _build_kernel_impl(
        L=L, warmup=13, preload_groups=((0, 1, 2, 3), (4, 5, 6, 7)))


def _shard_inputs(x, state, erase_kernel, erase_bias, write_kernel, write_bias):
    import ml_dtypes
    bf16 = ml_dtypes.bfloat16

    def pack_k(a2d):  # [DIN, C] -> [P, KT, C]
        return np.ascontiguousarray(
            a2d.reshape(KT, P, a2d.shape[1]).transpose(1, 0, 2))

    maps = []
    for core in range(8):
        b, h = divmod(core, 2)
        e0 = h * ESH
        xb = x[b]
        web = erase_kernel[:, e0:e0 + ESH]
        wwb = write_kernel[:, e0:e0 + ESH]
        if h == 1:
            xb = np.concatenate([xb[:, ESH:], xb[:, :ESH]], axis=1)
            web = np.concatenate([web[ESH:, :], web[:ESH, :]], axis=0)
            wwb = np.concatenate([wwb[ESH:, :], wwb[:ESH, :]], axis=0)
        ben = (-erase_bias[e0:e0 + ESH]).reshape(MT, P).T
        bwp = write_bias[e0:e0 + ESH].reshape(MT, P).T
        stp = state[b, e0:e0 + ESH].reshape(MT, P).T
        maps.append({
            "xT": pack_k(np.ascontiguousarray(xb.T).astype(bf16)),
            "we": pack_k(web.astype(bf16)),
            "ww": pack_k(wwb.astype(bf16)),
            "biases": np.ascontiguousarray(
                np.concatenate([ben, bwp], axis=1), dtype=np.float32),
            "state0": np.ascontiguousarray(stp, dtype=np.float32),
        })
    return maps


def kernel(x, state, erase_kernel, erase_bias, write_kernel, write_bias):
    global _cached_nc
    x = np.asarray(x, np.float32)
    state = np.asarray(state, np.float32)
    erase_kernel = np.asarray(erase_kernel, np.float32)
    erase_bias = np.asarray(erase_bias, np.float32)
    write_kernel = np.asarray(write_kernel, np.float32)
    write_bias = np.asarray(write_bias, np.float32)

    if _cached_nc is None:
        _cached_nc = _build_kernel()
    maps = _shard_inputs(x, state, erase_kernel, erase_bias,
                         write_kernel, write_bias)
    res = run_bass_kernel_spmd(_cached_nc, maps, core_ids=list(range(8)))
    full = np.empty((B, L, DIN), np.float32)
    for core in range(8):
        b, h = divmod(core, 2)
        o = res.results[core]["outT"]  # [P, MT, L]
        full[b, :, h * ESH:(h + 1) * ESH] = \
            o.transpose(2, 1, 0).reshape(L, ESH)
    return full


# revision 32
# speedup vs baseline: 1.1599x; 1.1194x over previous
"""DeltaTokenShift Trainium2 kernel (Bass/Tile, 8 NeuronCores via axon).

Computation (per batch b):
    erase = sigmoid(x @ We + be) ; write = sigmoid(x @ Ww + bw)
    s_t = s_{t-1} * (1 - erase_t) + write_t * x_t   (scan over L, per channel)
    out[:, t, :] = s_t

Sharding: 8 cores = 4 batches x 2 halves of the 1024-channel dim. Each core
gets the full x[b] (the gate matmul contracts over all 1024 input channels),
its 512-column weight slices, bias/state slices, and computes
out[b][:, half] = [4096, 512]. For upper-half cores, x columns and weight
rows are rotated by 512 on the host so the core's own gate channels always
occupy xT k-tiles 0..3 (a consistent permutation of the contraction dim
leaves the matmul result unchanged).

All layout work is host-side. x ships transposed, bf16, and packed in
k-planes: xT[p, k, l] = x[l, 128k+p] (so the contraction dim sits on
partitions with no PE transposes, and one DMA issue can move any k-range).
Weights likewise pack to [128, k, e] bf16; the kernel writes
outT[p, m, l] = out[l, 128m+p], unpacked on the host. bf16 gate inputs
keep the PE at 1 col/cycle (same as f32r) while halving HBM traffic;
fp8 was measured out of tolerance (2.6e-2) and bf16 in (3.3e-3).

The PE then runs ONLY the 512 gate matmuls [128e, 512l] = 8-step bf16
accumulations in PSUM, which is the compute floor for this op. Per
512-token chunk, per m-group: ACT sigmoid straight from PSUM (erase uses
scale=-1, bias=-be => a = 1-sigmoid), GpSimd b = write * x_f32 (x cast
hoisted to block top on DVE), DVE tensor_tensor_scan(a, b) chained via
initial=prev[:, -1:], per-m DMA of outT block slabs.

Ramp/tail details: 13 warmup matmuls on an iota-filled tile bridge the
DMA preload so the PE DVFS ladder (~0.8 -> ~1.35 -> 2.4GHz, ~3.4us of
continuous busy per step; zeros do NOT ramp it - no switching activity)
is climbed before real work; the preload is 2 k-group DMA issues per
tensor (per-issue cost ~650ns makes fine-grained preloads issue-bound);
the first chunk is erase-gate-major (write weights still streaming); the
last chunk is write-gate-major with b=w*x precomputed, so the post-last-
matmul chain is only sigmoid -> scan -> DMA.

Measured: ~133-136us HW exec (machine has a fast and a ~1.2x slower
whole-chip clock state), rel err 3.3e-3 vs fp32 CPU reference.
"""

import sys

sys.path.insert(0, "/opt/trn_rl_repo")

import numpy as np
import concourse.bacc as bacc
import concourse.mybir as mybir
from concourse.tile import TileContext
from concourse.bass_utils import run_bass_kernel_spmd

B, L = 4, 4096

F32 = mybir.dt.float32
F32R = mybir.dt.float32r
BF16 = mybir.dt.bfloat16

P = 128
DIN = 1024
ESH = 512
KT = DIN // P  # 8 contraction k-tiles
MT = ESH // P  # 4 output-channel groups per core


def _build_kernel_impl(L=4096, blocks=None, warmup=0, psum_bufs=4,
                       out_bf16=False, kmajor_first=False,
                       preload_groups=((0, 1, 2, 3), (4, 5)),
                       w_queue="sync", x_queue="sync", o_queue="sync"):
    """blocks: list of lists of chunk widths; each inner list is one DMA
    block (xT slab in, outT slab out). preload_groups: k-ranges, one DMA
    issue each, for the weight + block-0 x preload."""
    assert not kmajor_first, "kmajor_first path predates the fp8 tail"
    if blocks is None:
        blocks = [[512], [512, 512], [512, 512], [512, 512], [512]]
    assert sum(sum(b) for b in blocks) == L
    lbmax = max(sum(b) for b in blocks)
    assert sorted(k for g in preload_groups for k in g) == list(range(KT - 2))
    for g in preload_groups:
        assert list(g) == list(range(g[0], g[0] + len(g)))

    nc = bacc.Bacc("TRN2", target_bir_lowering=False)
    queues = {"sync": nc.sync, "scalar": nc.scalar, "vector": nc.vector,
              "gpsimd": nc.gpsimd}
    wq, xq, oq = queues[w_queue], queues[x_queue], queues[o_queue]

    KB = KT - 2  # bf16 k-planes
    xT = nc.dram_tensor("xT", [P, KB, L], BF16, kind="ExternalInput")
    FP8 = mybir.dt.float8e4
    K8 = 2  # k-tiles 6,7 run as one fp8 DoubleRow matmul
    xT8 = nc.dram_tensor("xT8", [P, K8, L], FP8, kind="ExternalInput")
    we8 = nc.dram_tensor("we8", [P, K8, ESH], FP8, kind="ExternalInput")
    ww8 = nc.dram_tensor("ww8", [P, K8, ESH], FP8, kind="ExternalInput")
    we = nc.dram_tensor("we", [P, KB, ESH], BF16, kind="ExternalInput")
    ww = nc.dram_tensor("ww", [P, KB, ESH], BF16, kind="ExternalInput")
    # biases[:, m] = -erase_bias group m ; biases[:, MT+m] = +write_bias group m
    biases = nc.dram_tensor("biases", [P, 2 * MT], F32, kind="ExternalInput")
    state0 = nc.dram_tensor("state0", [P, MT], F32, kind="ExternalInput")
    odt = BF16 if out_bf16 else F32
    outT = nc.dram_tensor("outT", [P, MT, L], odt, kind="ExternalOutput")

    with TileContext(nc) as tc:
        with (
            tc.tile_pool(name="const", bufs=1) as constp,
            tc.tile_pool(name="wsb", bufs=1) as wsb,
            tc.tile_pool(name="xt", bufs=2) as xtp,
            tc.tile_pool(name="gate", bufs=4) as gatep,
            tc.tile_pool(name="bmul", bufs=3) as bmulp,
            tc.tile_pool(name="scan", bufs=2) as scanp,
            tc.tile_pool(name="ps_mm", bufs=psum_bufs, space="PSUM") as ps_mm,
        ):
            if warmup:
                # Dummy matmuls keep the PE busy while the first DMAs
                # land. Random operand data (not zeros): the DVFS governor
                # responds to switching activity, so zero-valued warmups
                # leave the clock low.
                wu_sb = constp.tile([P, ESH], mybir.dt.int16, tag="wu")
                nc.gpsimd.iota(wu_sb[:], [[1, ESH]], channel_multiplier=997,
                               allow_small_or_imprecise_dtypes=True)
                wu_ps = [ps_mm.tile([P, ESH], F32, tag=f"wups{j}",
                                    name=f"wups{j}", bufs=1)
                         for j in range(2)]
                wu_b = wu_sb[:].bitcast(BF16)
                for j in range(warmup):
                    nc.tensor.matmul(
                        wu_ps[j % 2][:], wu_b[:, :P], wu_b,
                        start=True, stop=True, skip_group_check=True)

            # Preload: erase weights + block-0 x first (the first chunk is
            # emitted gate-major, so the PE only needs we+x to start), then
            # the write weights, which land during the erase sweep.
            lb0 = sum(blocks[0])
            w_sb = [wsb.tile([P, KB, ESH], BF16, tag=f"w{gi}", name=f"w{gi}")
                    for gi in range(2)]
            xt0 = xtp.tile([P, KB, lbmax], BF16, tag="xt", name="xt")
            w8_sb = [wsb.tile([P, K8, ESH], FP8, tag=f"w8{gi}",
                              name=f"w8{gi}") for gi in range(2)]
            xt8_0 = xtp.tile([P, K8, lbmax], FP8, tag="xt8", name="xt8")
            if kmajor_first:
                # k-major first chunk consumes both gates' k-slabs in
                # stream order, so interleave ww per group too.
                for g in preload_groups:
                    ks = slice(g[0], g[-1] + 1)
                    wq.dma_start(w_sb[0][:, ks, :], we[:, ks, :])
                    wq.dma_start(w_sb[1][:, ks, :], ww[:, ks, :])
                    xq.dma_start(xt0[:, ks, :lb0], xT[:, ks, :lb0])
            else:
                for gidx, g in enumerate(preload_groups):
                    ks = slice(g[0], g[-1] + 1)
                    wq.dma_start(w_sb[0][:, ks, :], we[:, ks, :])
                    xq.dma_start(xt0[:, ks, :lb0], xT[:, ks, :lb0])
                    if gidx == 0:
                        wq.dma_start(w8_sb[0][:], we8[:])
                        xq.dma_start(xt8_0[:, :, :lb0], xT8[:, :, :lb0])
                for gidx, g in enumerate(preload_groups):
                    ks = slice(g[0], g[-1] + 1)
                    wq.dma_start(w_sb[1][:, ks, :], ww[:, ks, :])
                    if gidx == 0:
                        wq.dma_start(w8_sb[1][:], ww8[:])

            bias_sb = constp.tile([P, 2 * MT], F32, tag="bias")
            nc.sync.dma_start(bias_sb[:], biases[:])
            st_sb = constp.tile([P, MT], F32, tag="st")
            nc.sync.dma_start(st_sb[:], state0[:])

            prev_s = [None] * MT
            b0 = 0

            for blki, chunks in enumerate(blocks):
                lb = sum(chunks)
                if blki == 0:
                    xt = xt0
                    xt8 = xt8_0
                else:
                    xt = xtp.tile([P, KB, lbmax], BF16, tag="xt", name="xt")
                    xq.dma_start(xt[:, :, :lb], xT[:, :, b0:b0 + lb])
                    xt8 = xtp.tile([P, K8, lbmax], FP8, tag="xt8",
                                   name="xt8")
                    xq.dma_start(xt8[:, :, :lb], xT8[:, :, b0:b0 + lb])

                # Hoist the f32 casts of the b-term x slabs out of the
                # per-m critical chain: they only depend on the xT DMA.
                xf = [None] * MT
                for m in range(MT):
                    xf[m] = bmulp.tile([P, lbmax], F32, tag=f"xf{m}",
                                       name=f"xf{m}", bufs=2)
                    nc.vector.tensor_copy(xf[m][:, :lb], xt[:, m, :lb])

                def mm_group(gi, m, lo, lc):
                    ps = ps_mm.tile([P, ESH], F32, tag="psmm", name="psmm")
                    for k in range(KB):
                        nc.tensor.matmul(
                            ps[:, :lc],
                            w_sb[gi][:, k, m * P:(m + 1) * P],
                            xt[:, k, lo:lo + lc],
                            start=(k == 0), stop=False,
                        )
                    nc.tensor.matmul(
                        ps[:, :lc],
                        w8_sb[gi][:, :, m * P:(m + 1) * P],
                        xt8[:, :, lo:lo + lc],
                        start=False, stop=True,
                        perf_mode=mybir.MatmulPerfMode.DoubleRow,
                    )
                    g_t = gatep.tile([P, ESH], F32, tag="aw"[gi],
                                     name="aw"[gi])
                    nc.scalar.activation(
                        g_t[:, :lc], ps[:, :lc],
                        mybir.ActivationFunctionType.Sigmoid,
                        bias=bias_sb[:, gi * MT + m:gi * MT + m + 1],
                        scale=-1.0 if gi == 0 else 1.0,
                    )
                    return g_t

                s_tiles = [None] * MT
                lo = 0
                for ci, lc in enumerate(chunks):
                    last_chunk = (blki == len(blocks) - 1
                                  and ci == len(chunks) - 1)
                    gates = [[None] * MT, [None] * MT]
                    if blki == 0 and ci == 0 and kmajor_first:
                        # k-major in two halves of 4 PSUM groups each: the
                        # PE consumes every k-slab as it streams in instead
                        # of head-of-line blocking on a later k while an
                        # earlier one still has runnable matmuls.
                        for ms in ((0, 1), (2, 3)):
                            pss = {}
                            for m in ms:
                                for gi in range(2):
                                    pss[(gi, m)] = ps_mm.tile(
                                        [P, ESH], F32, tag="psmm",
                                        name="psmm")
                            for k in range(KT):
                                for m in ms:
                                    for gi in range(2):
                                        nc.tensor.matmul(
                                            pss[(gi, m)][:, :lc],
                                            w_sb[gi][:, k,
                                                     m * P:(m + 1) * P],
                                            xt[:, k, lo:lo + lc],
                                            start=(k == 0),
                                            stop=(k == KT - 1),
                                        )
                            for m in ms:
                                for gi in range(2):
                                    g_t = gatep.tile(
                                        [P, ESH], F32, tag="aw"[gi],
                                        name="aw"[gi])
                                    nc.scalar.activation(
                                        g_t[:, :lc], pss[(gi, m)][:, :lc],
                                        mybir.ActivationFunctionType.Sigmoid,
                                        bias=bias_sb[
                                            :, gi * MT + m:gi * MT + m + 1],
                                        scale=-1.0 if gi == 0 else 1.0,
                                    )
                                    gates[gi][m] = g_t
                    elif blki == 0 and ci == 0:
                        # Gate-major: the erase groups only need the we
                        # tiles, which land first during the preload.
                        for gi in range(2):
                            for m in range(MT):
                                gates[gi][m] = mm_group(gi, m, lo, lc)
                    elif last_chunk:
                        # Write-gate-major: the b = w*x mults complete
                        # while the erase groups still run, so the tail
                        # chain after the final matmul is only
                        # sigmoid -> scan -> DMA.
                        for gi in (1, 0):
                            for m in range(MT):
                                gates[gi][m] = mm_group(gi, m, lo, lc)
                        for m in range(MT):
                            b_t = bmulp.tile([P, ESH], F32, tag="b")
                            nc.gpsimd.tensor_tensor(
                                b_t[:, :lc], gates[1][m][:, :lc],
                                xf[m][:, lo:lo + lc],
                                op=mybir.AluOpType.mult)
                            gates[1][m] = b_t
                    else:
                        for m in range(MT):
                            gates[0][m] = mm_group(0, m, lo, lc)
                            gates[1][m] = mm_group(1, m, lo, lc)

                    for m in range(MT):
                        a_t = gates[0][m]
                        if last_chunk:
                            b_t = gates[1][m]  # already w*x
                        else:
                            b_t = bmulp.tile([P, ESH], F32, tag="b")
                            # GpSimd is otherwise idle; fully parallel with
                            # DVE, and operands + out are SBUF (P2-safe).
                            nc.gpsimd.tensor_tensor(
                                b_t[:, :lc], gates[1][m][:, :lc],
                                xf[m][:, lo:lo + lc],
                                op=mybir.AluOpType.mult)

                        if ci == 0:
                            s_tiles[m] = scanp.tile(
                                [P, lbmax], odt, tag=f"s{m}", name=f"s{m}")
                            init = st_sb[:, m:m + 1] if blki == 0 else \
                                prev_s[m][:, prev_lb - 1:prev_lb]
                        else:
                            init = s_tiles[m][:, lo - 1:lo]
                        nc.vector.tensor_tensor_scan(
                            s_tiles[m][:, lo:lo + lc], a_t[:, :lc],
                            b_t[:, :lc], init,
                            op0=mybir.AluOpType.mult, op1=mybir.AluOpType.add,
                        )
                        if ci == len(chunks) - 1:
                            oq.dma_start(
                                outT[:, m, b0:b0 + lb], s_tiles[m][:, :lb])
                            prev_s[m] = s_tiles[m]
                    lo += lc
                prev_lb = lb
                b0 += lb

    nc.finalize()
    return nc


_cached_nc = None


def _build_kernel():
    return _build_kernel_impl(
        L=L, warmup=13, preload_groups=((0, 1, 2, 3), (4, 5)))


def _shard_inputs(x, state, erase_kernel, erase_bias, write_kernel, write_bias):
    import ml_dtypes
    bf16 = ml_dtypes.bfloat16

    def pack_k(a2d):  # [DIN, C] -> [P, KT, C]
        return np.ascontiguousarray(
            a2d.reshape(KT, P, a2d.shape[1]).transpose(1, 0, 2))

    maps = []
    for core in range(8):
        b, h = divmod(core, 2)
        e0 = h * ESH
        xb = x[b]
        web = erase_kernel[:, e0:e0 + ESH]
        wwb = write_kernel[:, e0:e0 + ESH]
        if h == 1:
            xb = np.concatenate([xb[:, ESH:], xb[:, :ESH]], axis=1)
            web = np.concatenate([web[ESH:, :], web[:ESH, :]], axis=0)
            wwb = np.concatenate([wwb[ESH:, :], wwb[:ESH, :]], axis=0)
        ben = (-erase_bias[e0:e0 + ESH]).reshape(MT, P).T
        bwp = write_bias[e0:e0 + ESH].reshape(MT, P).T
        stp = state[b, e0:e0 + ESH].reshape(MT, P).T
        f8 = ml_dtypes.float8_e4m3
        xbT = np.ascontiguousarray(xb.T)

        def pack8(a2d):  # [256, C] fp8 tail -> [P, 2, C]
            return np.ascontiguousarray(
                a2d.reshape(2, P, a2d.shape[1]).transpose(1, 0, 2))

        maps.append({
            "xT": np.ascontiguousarray(
                pack_k(xbT.astype(bf16))[:, :6, :]),
            "xT8": pack8((xbT[768:1024] / 4).astype(f8)),
            "we": np.ascontiguousarray(pack_k(web.astype(bf16))[:, :6, :]),
            "ww": np.ascontiguousarray(pack_k(wwb.astype(bf16))[:, :6, :]),
            "we8": pack8((web[768:1024] * 4).astype(f8)),
            "ww8": pack8((wwb[768:1024] * 4).astype(f8)),
            "biases": np.ascontiguousarray(
                np.concatenate([ben, bwp], axis=1), dtype=np.float32),
            "state0": np.ascontiguousarray(stp, dtype=np.float32),
        })
    return maps


def kernel(x, state, erase_kernel, erase_bias, write_kernel, write_bias):
    global _cached_nc
    x = np.asarray(x, np.float32)
    state = np.asarray(state, np.float32)
    erase_kernel = np.asarray(erase_kernel, np.float32)
    erase_bias = np.asarray(erase_bias, np.float32)
    write_kernel = np.asarray(write_kernel, np.float32)
    write_bias = np.asarray(write_bias, np.float32)

    if _cached_nc is None:
        _cached_nc = _build_kernel()
    maps = _shard_inputs(x, state, erase_kernel, erase_bias,
                         write_kernel, write_bias)
    res = run_bass_kernel_spmd(_cached_nc, maps, core_ids=list(range(8)))
    full = np.empty((B, L, DIN), np.float32)
    for core in range(8):
        b, h = divmod(core, 2)
        o = res.results[core]["outT"]  # [P, MT, L]
        full[b, :, h * ESH:(h + 1) * ESH] = \
            o.transpose(2, 1, 0).reshape(L, ESH)
    return full


# revision 34
# speedup vs baseline: 1.2159x; 1.0483x over previous
"""DeltaTokenShift Trainium2 kernel (Bass/Tile, 8 NeuronCores via axon).

Computation (per batch b):
    erase = sigmoid(x @ We + be) ; write = sigmoid(x @ Ww + bw)
    s_t = s_{t-1} * (1 - erase_t) + write_t * x_t   (scan over L, per channel)
    out[:, t, :] = s_t

Sharding: 8 cores = 4 batches x 2 halves of the 1024-channel dim. Each core
gets the full x[b] (the gate matmul contracts over all 1024 input channels),
its 512-column weight slices, bias/state slices, and computes
out[b][:, half] = [4096, 512]. For upper-half cores, x columns and weight
rows are rotated by 512 on the host so the core's own gate channels always
occupy xT k-tiles 0..3 (a consistent permutation of the contraction dim
leaves the matmul result unchanged).

All layout work is host-side. x ships transposed, bf16, and packed in
k-planes: xT[p, k, l] = x[l, 128k+p] (so the contraction dim sits on
partitions with no PE transposes, and one DMA issue can move any k-range).
Weights likewise pack to [128, k, e] bf16; the kernel writes
outT[p, m, l] = out[l, 128m+p], unpacked on the host. bf16 gate inputs
keep the PE at 1 col/cycle (same as f32r) while halving HBM traffic.

Hybrid precision on the contraction: k-tiles 0..5 run in bf16, k-tiles
6..7 run as ONE fp8e4m3 DoubleRow matmul (0.5 cyc/row, operands packed
[128, 2, n], x/4 and W*4 so the product needs no descale) closing each
PSUM accumulation. Full-fp8 fails the 2e-2 gate (2.6e-2); the 1/4-fp8
split lands at 1.36e-2 (error scales with sqrt of the fp8 fraction)
while cutting PE time 12.5%.

The PE then runs 512 gate matmul groups [128e, 512l] = 6 bf16 + 1 fp8-DR
accumulation steps in PSUM, which is the compute floor at this precision
split. Per
512-token chunk, per m-group: ACT sigmoid straight from PSUM (erase uses
scale=-1, bias=-be => a = 1-sigmoid), GpSimd b = write * x_f32 (x cast
hoisted to block top on DVE), DVE tensor_tensor_scan(a, b) chained via
initial=prev[:, -1:], per-m DMA of outT block slabs.

Ramp/tail details: 13 warmup matmuls on an iota-filled tile bridge the
DMA preload so the PE DVFS ladder (~0.8 -> ~1.35 -> 2.4GHz, ~3.4us of
continuous busy per step; zeros do NOT ramp it - no switching activity)
is climbed before real work; the preload is 2 k-group DMA issues per
tensor (per-issue cost ~650ns makes fine-grained preloads issue-bound);
the first chunk is erase-gate-major (write weights still streaming); the
last chunk is write-gate-major with b=w*x precomputed, so the post-last-
matmul chain is only sigmoid -> scan -> DMA.

Measured: ~120us HW exec (best 119.7; machine has a fast and a ~1.2x
slower whole-chip clock state), rel err 1.358e-2 vs fp32 CPU reference
(deterministic: setup_inputs is seeded).
"""

import sys

sys.path.insert(0, "/opt/trn_rl_repo")

import numpy as np
import concourse.bacc as bacc
import concourse.mybir as mybir
from concourse.tile import TileContext
from concourse.bass_utils import run_bass_kernel_spmd

B, L = 4, 4096

F32 = mybir.dt.float32
F32R = mybir.dt.float32r
BF16 = mybir.dt.bfloat16

P = 128
DIN = 1024
ESH = 512
KT = DIN // P  # 8 contraction k-tiles
MT = ESH // P  # 4 output-channel groups per core


def _build_kernel_impl(L=4096, blocks=None, warmup=0, psum_bufs=4,
                       out_bf16=False, kmajor_first=False,
                       preload_groups=((0, 1, 2, 3), (4, 5)),
                       w_queue="sync", x_queue="sync", o_queue="sync"):
    """blocks: list of lists of chunk widths; each inner list is one DMA
    block (xT slab in, outT slab out). preload_groups: k-ranges, one DMA
    issue each, for the weight + block-0 x preload."""
    assert not kmajor_first, "kmajor_first path predates the fp8 tail"
    if blocks is None:
        blocks = [[512], [512, 512], [512, 512], [512, 512], [512]]
    assert sum(sum(b) for b in blocks) == L
    lbmax = max(sum(b) for b in blocks)
    assert sorted(k for g in preload_groups for k in g) == list(range(KT - 2))
    for g in preload_groups:
        assert list(g) == list(range(g[0], g[0] + len(g)))

    nc = bacc.Bacc("TRN2", target_bir_lowering=False)
    queues = {"sync": nc.sync, "scalar": nc.scalar, "vector": nc.vector,
              "gpsimd": nc.gpsimd}
    wq, xq, oq = queues[w_queue], queues[x_queue], queues[o_queue]

    KB = KT - 2  # bf16 k-planes
    xT = nc.dram_tensor("xT", [P, KB, L], BF16, kind="ExternalInput")
    FP8 = mybir.dt.float8e4
    K8 = (4, 2)  # fp8 k-tiles per gate: erase k4-7 (2 DR), write k6-7 (1 DR)
    xT8 = nc.dram_tensor("xT8", [P, 4, L], FP8, kind="ExternalInput")
    we8 = nc.dram_tensor("we8", [P, 4, ESH], FP8, kind="ExternalInput")
    ww8 = nc.dram_tensor("ww8", [P, 2, ESH], FP8, kind="ExternalInput")
    we = nc.dram_tensor("we", [P, KB, ESH], BF16, kind="ExternalInput")
    ww = nc.dram_tensor("ww", [P, KB, ESH], BF16, kind="ExternalInput")
    # biases[:, m] = -erase_bias group m ; biases[:, MT+m] = +write_bias group m
    biases = nc.dram_tensor("biases", [P, 2 * MT], F32, kind="ExternalInput")
    state0 = nc.dram_tensor("state0", [P, MT], F32, kind="ExternalInput")
    odt = BF16 if out_bf16 else F32
    outT = nc.dram_tensor("outT", [P, MT, L], odt, kind="ExternalOutput")

    with TileContext(nc) as tc:
        with (
            tc.tile_pool(name="const", bufs=1) as constp,
            tc.tile_pool(name="wsb", bufs=1) as wsb,
            tc.tile_pool(name="xt", bufs=2) as xtp,
            tc.tile_pool(name="gate", bufs=4) as gatep,
            tc.tile_pool(name="bmul", bufs=3) as bmulp,
            tc.tile_pool(name="scan", bufs=2) as scanp,
            tc.tile_pool(name="ps_mm", bufs=psum_bufs, space="PSUM") as ps_mm,
        ):
            if warmup:
                # Dummy matmuls keep the PE busy while the first DMAs
                # land. Random operand data (not zeros): the DVFS governor
                # responds to switching activity, so zero-valued warmups
                # leave the clock low.
                wu_sb = constp.tile([P, ESH], mybir.dt.int16, tag="wu")
                nc.gpsimd.iota(wu_sb[:], [[1, ESH]], channel_multiplier=997,
                               allow_small_or_imprecise_dtypes=True)
                wu_ps = [ps_mm.tile([P, ESH], F32, tag=f"wups{j}",
                                    name=f"wups{j}", bufs=1)
                         for j in range(2)]
                wu_b = wu_sb[:].bitcast(BF16)
                for j in range(warmup):
                    nc.tensor.matmul(
                        wu_ps[j % 2][:], wu_b[:, :P], wu_b,
                        start=True, stop=True, skip_group_check=True)

            # Preload: erase weights + block-0 x first (the first chunk is
            # emitted gate-major, so the PE only needs we+x to start), then
            # the write weights, which land during the erase sweep.
            lb0 = sum(blocks[0])
            w_sb = [wsb.tile([P, KB, ESH], BF16, tag=f"w{gi}", name=f"w{gi}")
                    for gi in range(2)]
            xt0 = xtp.tile([P, KB, lbmax], BF16, tag="xt", name="xt")
            w8_sb = [wsb.tile([P, K8[gi], ESH], FP8, tag=f"w8{gi}",
                              name=f"w8{gi}") for gi in range(2)]
            xt8_0 = xtp.tile([P, 4, lbmax], FP8, tag="xt8", name="xt8")
            if kmajor_first:
                # k-major first chunk consumes both gates' k-slabs in
                # stream order, so interleave ww per group too.
                for g in preload_groups:
                    ks = slice(g[0], g[-1] + 1)
                    wq.dma_start(w_sb[0][:, ks, :], we[:, ks, :])
                    wq.dma_start(w_sb[1][:, ks, :], ww[:, ks, :])
                    xq.dma_start(xt0[:, ks, :lb0], xT[:, ks, :lb0])
            else:
                for gidx, g in enumerate(preload_groups):
                    ks = slice(g[0], g[-1] + 1)
                    wq.dma_start(w_sb[0][:, ks, :], we[:, ks, :])
                    xq.dma_start(xt0[:, ks, :lb0], xT[:, ks, :lb0])
                    if gidx == 0:
                        wq.dma_start(w8_sb[0][:], we8[:])
                        xq.dma_start(xt8_0[:, :, :lb0], xT8[:, :, :lb0])
                for gidx, g in enumerate(preload_groups):
                    ks = slice(g[0], g[-1] + 1)
                    wq.dma_start(w_sb[1][:, ks, :], ww[:, ks, :])
                    if gidx == 0:
                        wq.dma_start(w8_sb[1][:], ww8[:])

            bias_sb = constp.tile([P, 2 * MT], F32, tag="bias")
            nc.sync.dma_start(bias_sb[:], biases[:])
            st_sb = constp.tile([P, MT], F32, tag="st")
            nc.sync.dma_start(st_sb[:], state0[:])

            prev_s = [None] * MT
            b0 = 0

            for blki, chunks in enumerate(blocks):
                lb = sum(chunks)
                if blki == 0:
                    xt = xt0
                    xt8 = xt8_0
                else:
                    xt = xtp.tile([P, KB, lbmax], BF16, tag="xt", name="xt")
                    xq.dma_start(xt[:, :, :lb], xT[:, :, b0:b0 + lb])
                    xt8 = xtp.tile([P, 4, lbmax], FP8, tag="xt8",
                                   name="xt8")
                    xq.dma_start(xt8[:, :, :lb], xT8[:, :, b0:b0 + lb])

                # Hoist the f32 casts of the b-term x slabs out of the
                # per-m critical chain: they only depend on the xT DMA.
                xf = [None] * MT
                for m in range(MT):
                    xf[m] = bmulp.tile([P, lbmax], F32, tag=f"xf{m}",
                                       name=f"xf{m}", bufs=2)
                    nc.vector.tensor_copy(xf[m][:, :lb], xt[:, m, :lb])

                def mm_group(gi, m, lo, lc):
                    ps = ps_mm.tile([P, ESH], F32, tag="psmm", name="psmm")
                    kb = KT - K8[gi]
                    for k in range(kb):
                        nc.tensor.matmul(
                            ps[:, :lc],
                            w_sb[gi][:, k, m * P:(m + 1) * P],
                            xt[:, k, lo:lo + lc],
                            start=(k == 0), stop=False,
                        )
                    ndr = K8[gi] // 2
                    for j in range(ndr):
                        x8lo = 2 - ndr + j  # xT8 plane pair for k = kb+2j
                        nc.tensor.matmul(
                            ps[:, :lc],
                            w8_sb[gi][:, 2 * j:2 * j + 2,
                                      m * P:(m + 1) * P],
                            xt8[:, 2 * x8lo:2 * x8lo + 2, lo:lo + lc],
                            start=False, stop=(j == ndr - 1),
                            perf_mode=mybir.MatmulPerfMode.DoubleRow,
                        )
                    g_t = gatep.tile([P, ESH], F32, tag="aw"[gi],
                                     name="aw"[gi])
                    nc.scalar.activation(
                        g_t[:, :lc], ps[:, :lc],
                        mybir.ActivationFunctionType.Sigmoid,
                        bias=bias_sb[:, gi * MT + m:gi * MT + m + 1],
                        scale=-1.0 if gi == 0 else 1.0,
                    )
                    return g_t

                s_tiles = [None] * MT
                lo = 0
                for ci, lc in enumerate(chunks):
                    last_chunk = (blki == len(blocks) - 1
                                  and ci == len(chunks) - 1)
                    gates = [[None] * MT, [None] * MT]
                    if blki == 0 and ci == 0 and kmajor_first:
                        # k-major in two halves of 4 PSUM groups each: the
                        # PE consumes every k-slab as it streams in instead
                        # of head-of-line blocking on a later k while an
                        # earlier one still has runnable matmuls.
                        for ms in ((0, 1), (2, 3)):
                            pss = {}
                            for m in ms:
                                for gi in range(2):
                                    pss[(gi, m)] = ps_mm.tile(
                                        [P, ESH], F32, tag="psmm",
                                        name="psmm")
                            for k in range(KT):
                                for m in ms:
                                    for gi in range(2):
                                        nc.tensor.matmul(
                                            pss[(gi, m)][:, :lc],
                                            w_sb[gi][:, k,
                                                     m * P:(m + 1) * P],
                                            xt[:, k, lo:lo + lc],
                                            start=(k == 0),
                                            stop=(k == KT - 1),
                                        )
                            for m in ms:
                                for gi in range(2):
                                    g_t = gatep.tile(
                                        [P, ESH], F32, tag="aw"[gi],
                                        name="aw"[gi])
                                    nc.scalar.activation(
                                        g_t[:, :lc], pss[(gi, m)][:, :lc],
                                        mybir.ActivationFunctionType.Sigmoid,
                                        bias=bias_sb[
                                            :, gi * MT + m:gi * MT + m + 1],
                                        scale=-1.0 if gi == 0 else 1.0,
                                    )
                                    gates[gi][m] = g_t
                    elif blki == 0 and ci == 0:
                        # Gate-major: the erase groups only need the we
                        # tiles, which land first during the preload.
                        for gi in range(2):
                            for m in range(MT):
                                gates[gi][m] = mm_group(gi, m, lo, lc)
                    elif last_chunk:
                        # Write-gate-major: the b = w*x mults complete
                        # while the erase groups still run, so the tail
                        # chain after the final matmul is only
                        # sigmoid -> scan -> DMA.
                        for gi in (1, 0):
                            for m in range(MT):
                                gates[gi][m] = mm_group(gi, m, lo, lc)
                        for m in range(MT):
                            b_t = bmulp.tile([P, ESH], F32, tag="b")
                            nc.gpsimd.tensor_tensor(
                                b_t[:, :lc], gates[1][m][:, :lc],
                                xf[m][:, lo:lo + lc],
                                op=mybir.AluOpType.mult)
                            gates[1][m] = b_t
                    else:
                        for m in range(MT):
                            gates[0][m] = mm_group(0, m, lo, lc)
                            gates[1][m] = mm_group(1, m, lo, lc)

                    for m in range(MT):
                        a_t = gates[0][m]
                        if last_chunk:
                            b_t = gates[1][m]  # already w*x
                        else:
                            b_t = bmulp.tile([P, ESH], F32, tag="b")
                            # GpSimd is otherwise idle; fully parallel with
                            # DVE, and operands + out are SBUF (P2-safe).
                            nc.gpsimd.tensor_tensor(
                                b_t[:, :lc], gates[1][m][:, :lc],
                                xf[m][:, lo:lo + lc],
                                op=mybir.AluOpType.mult)

                        if ci == 0:
                            s_tiles[m] = scanp.tile(
                                [P, lbmax], odt, tag=f"s{m}", name=f"s{m}")
                            init = st_sb[:, m:m + 1] if blki == 0 else \
                                prev_s[m][:, prev_lb - 1:prev_lb]
                        else:
                            init = s_tiles[m][:, lo - 1:lo]
                        nc.vector.tensor_tensor_scan(
                            s_tiles[m][:, lo:lo + lc], a_t[:, :lc],
                            b_t[:, :lc], init,
                            op0=mybir.AluOpType.mult, op1=mybir.AluOpType.add,
                        )
                        if ci == len(chunks) - 1:
                            oq.dma_start(
                                outT[:, m, b0:b0 + lb], s_tiles[m][:, :lb])
                            prev_s[m] = s_tiles[m]
                    lo += lc
                prev_lb = lb
                b0 += lb

    nc.finalize()
    return nc


_cached_nc = None


def _build_kernel():
    return _build_kernel_impl(
        L=L, warmup=13, preload_groups=((0, 1, 2, 3), (4, 5)))


def _shard_inputs(x, state, erase_kernel, erase_bias, write_kernel, write_bias):
    import ml_dtypes
    bf16 = ml_dtypes.bfloat16

    def pack_k(a2d):  # [DIN, C] -> [P, KT, C]
        return np.ascontiguousarray(
            a2d.reshape(KT, P, a2d.shape[1]).transpose(1, 0, 2))

    maps = []
    for core in range(8):
        b, h = divmod(core, 2)
        e0 = h * ESH
        xb = x[b]
        web = erase_kernel[:, e0:e0 + ESH]
        wwb = write_kernel[:, e0:e0 + ESH]
        if h == 1:
            xb = np.concatenate([xb[:, ESH:], xb[:, :ESH]], axis=1)
            web = np.concatenate([web[ESH:, :], web[:ESH, :]], axis=0)
            wwb = np.concatenate([wwb[ESH:, :], wwb[:ESH, :]], axis=0)
        ben = (-erase_bias[e0:e0 + ESH]).reshape(MT, P).T
        bwp = write_bias[e0:e0 + ESH].reshape(MT, P).T
        stp = state[b, e0:e0 + ESH].reshape(MT, P).T
        f8 = ml_dtypes.float8_e4m3
        xbT = np.ascontiguousarray(xb.T)

        def pack8(a2d):  # [n*128, C] fp8 tail -> [P, n, C]
            n = a2d.shape[0] // P
            return np.ascontiguousarray(
                a2d.reshape(n, P, a2d.shape[1]).transpose(1, 0, 2))

        maps.append({
            "xT": np.ascontiguousarray(
                pack_k(xbT.astype(bf16))[:, :6, :]),
            "xT8": pack8((xbT[512:1024] / 4).astype(f8)),
            "we": np.ascontiguousarray(pack_k(web.astype(bf16))[:, :6, :]),
            "ww": np.ascontiguousarray(pack_k(wwb.astype(bf16))[:, :6, :]),
            "we8": pack8((web[512:1024] * 4).astype(f8)),
            "ww8": pack8((wwb[768:1024] * 4).astype(f8)),
            "biases": np.ascontiguousarray(
                np.concatenate([ben, bwp], axis=1), dtype=np.float32),
            "state0": np.ascontiguousarray(stp, dtype=np.float32),
        })
    return maps


def kernel(x, state, erase_kernel, erase_bias, write_kernel, write_bias):
    global _cached_nc
    x = np.asarray(x, np.float32)
    state = np.asarray(state, np.float32)
    erase_kernel = np.asarray(erase_kernel, np.float32)
    erase_bias = np.asarray(erase_bias, np.float32)
    write_kernel = np.asarray(write_kernel, np.float32)
    write_bias = np.asarray(write_bias, np.float32)

    if _cached_nc is None:
        _cached_nc = _build_kernel()
    maps = _shard_inputs(x, state, erase_kernel, erase_bias,
                         write_kernel, write_bias)
    res = run_bass_kernel_spmd(_cached_nc, maps, core_ids=list(range(8)))
    full = np.empty((B, L, DIN), np.float32)
    for core in range(8):
        b, h = divmod(core, 2)
        o = res.results[core]["outT"]  # [P, MT, L]
        full[b, :, h * ESH:(h + 1) * ESH] = \
            o.transpose(2, 1, 0).reshape(L, ESH)
    return full
